# revision 4
# baseline (speedup 1.0000x reference)
"""Trainium2 Bass kernel: per-pixel 5x5 kernel application (KPN-style).

    out[b,c,y,x] = sum_{i,j} softmax(kernels[b,:,y,x])[i*5+j]
                   * zpad(data)[b,c,y+i,x+j]          (i,j in 0..4, r=2)

Sharding (8 NeuronCores, pure data parallel, no collectives):
    core = (b, H-half): 4 batches x 2 row-bands of 360 rows.
    Halo rows come from overlapping host-side slices of the full input.

Per-core algorithm (tiles live in "data space": 124 partitions =
120 output rows + 2 halo rows each side):
    - kernel taps arrive as 5 di-group DMAs per row-tile, rows shifted by
      -di; the 5 dj taps of a group are DRAM-contiguous, so each DMA is
      124 x 12.8KB descriptors (fat descriptors keep the software DGE
      descriptor-generation rate off the critical path).
    - exp runs IN-PLACE on the kernel tiles (ACT, bf16).
    - DVE forms tap-product planes q = e * d in bf16 2x mode; one
      instruction covers all even (or odd) dj taps of a di-group via an
      overlapping-window access pattern on the data tile. Two parity-
      aligned copies of the data (dbf0/dbf1) keep operands 4B-aligned.
    - PE folds the di row-shift into its stationary shift matrix
      S_di[k,m] = [k == m+di] and accumulates the 25 q planes per channel
      plus the 25 exp planes (softmax denominator) into PSUM; one matmul
      covers all 3 channel banks.
    - DVE: out_c = PSUM_c * reciprocal(PSUM_sum), written as bf16.

DMA (all SWDGE): HWDGE rings pin to 4 of the 16 SDMA engines (~22.5GB/s
each); the gpsimd software-DGE round-robins descriptors across all 16.
Stores are deferred to the next row-tile boundary to avoid head-of-line
blocking on the single SWDGE queue.

kernel(**inputs) takes the FULL inputs and returns the FULL output.
"""

import numpy as np
import ml_dtypes

B, C, H, W, KW = 4, 3, 720, 1280, 5
NCORES = 8
HS = H // 2            # 360 output rows per shard
RT = 120               # output rows per row-tile
NRT = HS // RT         # 3 row-tiles
HALO = 2
DP = RT + 2 * HALO     # 124 partitions (data space)
WP = 1288              # padded data width: 2 left + 1280 + 6 right
KROWPAD = 4            # zero rows around each kernel shard (top+bottom)
KH = HS + 2 * KROWPAD  # 368
XCH = [(0, 512), (512, 512), (1024, 256)]

MULTIBANK = False      # matmul out cannot cross PSUM bank boundaries

_CACHE = {}


def _build_program():
    import concourse.bacc as bacc
    import concourse.mybir as mybir
    from concourse.bass import AP
    from concourse import tile

    f32 = mybir.dt.float32
    bf16 = mybir.dt.bfloat16

    nc = bacc.Bacc(
        "TRN2",
        target_bir_lowering=False,
        debug=False,
        enable_asserts=False,
        num_devices=NCORES,
    )
    d_data = nc.dram_tensor("data", [HS + 2 * HALO, C, WP], bf16, kind="ExternalInput")
    d_kern = nc.dram_tensor("kern", [KH, KW * KW, W], bf16, kind="ExternalInput")
    d_out = nc.dram_tensor("out", [HS, C, W], bf16, kind="ExternalOutput")

    # Shift matrices S_di[k, m] = 1 iff k == m + di  (k: 124 data rows,
    # m: 120 out rows). Baked into the NEFF as a Const tensor.
    s_np = np.zeros((KW, DP, RT), dtype=ml_dtypes.bfloat16)
    for di in range(KW):
        for m in range(RT):
            s_np[di, m + di, m] = 1.0
    d_s = nc.inline_tensor(np.ascontiguousarray(s_np), "smat")

    KROW = KW * KW * W  # element stride between rows of d_kern

    with tile.TileContext(nc) as tc:
        with tc.tile_pool(name="const", bufs=1) as cpool, \
             tc.tile_pool(name="dbf", bufs=2) as dbfpool, \
             tc.tile_pool(name="eg", bufs=8) as egpool, \
             tc.tile_pool(name="qt", bufs=3) as qpool, \
             tc.tile_pool(name="fin", bufs=2) as fpool, \
             tc.tile_pool(name="ps", bufs=2, space="PSUM") as ppool:

            s_sb = cpool.tile([DP, KW, RT], bf16)
            nc.sync.dma_start(out=s_sb[:], in_=d_s.ap().transpose([1, 0, 2]))

            pending_store = []

            def flush_store():
                while pending_store:
                    yy, t = pending_store.pop()
                    nc.gpsimd.dma_start(out=d_out.ap()[yy:yy + RT], in_=t[:])

            for rt in range(NRT):
                y0 = rt * RT

                # kernel di-group tiles, rows shifted by -di:
                # et[di][p, dj, x] = kern[y0 + p - di, 5*di + dj, x]
                # (exp applied in place). One DMA per group: per-partition
                # 5 contiguous taps = 12.8KB descriptors.
                ets = []
                for di in range(KW):
                    et = egpool.tile([DP, KW, W], bf16, tag="eg")
                    off = (KROWPAD + y0 - di) * KROW + di * KW * W
                    nc.gpsimd.dma_start(
                        out=et[:],
                        in_=AP(d_kern, off, [[KROW, DP], [1, KW * W]]),
                    )
                    ets.append(et)
                    if di == 0:
                        # data rows y0-2 .. y0+121 (host-padded), bf16
                        dbf0 = dbfpool.tile([DP, C, WP], bf16, tag="dbf0")
                        dbf1 = dbfpool.tile([DP, C, WP], bf16, tag="dbf1")
                        nc.gpsimd.dma_start(
                            out=dbf0[:], in_=d_data.ap()[y0:y0 + DP],
                        )
                    nc.scalar.activation(
                        et[:], et[:], mybir.ActivationFunctionType.Exp,
                    )
                # dbf1 = dbf0 shifted one element left (odd-dj 4B alignment);
                # tensor_copy runs in DVE 4x mode.
                f0 = dbf0[:].rearrange("p c w -> p (c w)")
                f1 = dbf1[:].rearrange("p c w -> p (c w)")
                nc.vector.tensor_copy(f1[:, 0:C * WP - 1], f0[:, 1:C * WP])
                flush_store()

                rs = fpool.tile([RT, W], f32, tag="rs", bufs=1)
                ost = fpool.tile([RT, C, W], bf16, tag="ost")

                dbf0_ap = dbf0[:]
                dbf1_ap = dbf1[:]
                dp_stride = dbf0_ap.ap[0][0]

                for (xc, xcw) in XCH:
                    # PSUM banks: 0..2 = channel accumulators, 3 = sumexp
                    pacc = ppool.tile([RT, 4, 512], f32, tag="pacc")

                    for di in range(KW):
                        et = ets[di]
                        lhs = s_sb[:, di, :]
                        first = di == 0
                        last = di == KW - 1
                        # sumexp: stream the exp planes directly
                        for dj in range(KW):
                            nc.tensor.matmul(
                                out=pacc[:, 3, 0:xcw],
                                lhsT=lhs,
                                rhs=et[:, dj, xc:xc + xcw],
                                start=first and dj == 0,
                                stop=last and dj == KW - 1,
                            )
                        # tap products q[p, dj, c, x] = e[p, dj, x] *
                        # d[p, c, x + dj]; one DVE instruction per parity
                        # (overlapping dj windows, stride 2, on dbf0/dbf1).
                        qt = qpool.tile([DP, KW, C, 512], bf16, tag="qt")
                        e_ev = (
                            et[:, 0:KW:2, xc:xc + xcw]
                            .unsqueeze(2).broadcast_to([DP, 3, C, xcw])
                        )
                        d_ev = AP(
                            dbf0_ap.tensor,
                            dbf0_ap.offset + xc,
                            [[dp_stride, DP], [2, 3], [WP, C], [1, xcw]],
                        )
                        nc.vector.tensor_tensor(
                            qt[:, 0:KW:2, :, 0:xcw], e_ev, d_ev,
                            mybir.AluOpType.mult,
                        )
                        e_od = (
                            et[:, 1:KW:2, xc:xc + xcw]
                            .unsqueeze(2).broadcast_to([DP, 2, C, xcw])
                        )
                        d_od = AP(
                            dbf1_ap.tensor,
                            dbf1_ap.offset + xc,
                            [[dp_stride, DP], [2, 2], [WP, C], [1, xcw]],
                        )
                        nc.vector.tensor_tensor(
                            qt[:, 1:KW:2, :, 0:xcw], e_od, d_od,
                            mybir.AluOpType.mult,
                        )

                        for dj in range(KW):
                            if MULTIBANK:
                                nc.tensor.matmul(
                                    out=pacc[:, 0:3, 0:xcw],
                                    lhsT=lhs,
                                    rhs=qt[:, dj, :, 0:xcw],
                                    start=first and dj == 0,
                                    stop=last and dj == KW - 1,
                                )
                            else:
                                for c in range(C):
                                    nc.tensor.matmul(
                                        out=pacc[:, c, 0:xcw],
                                        lhsT=lhs,
                                        rhs=qt[:, dj, c, 0:xcw],
                                        start=first and dj == 0,
                                        stop=last and dj == KW - 1,
                                    )

                    nc.vector.reciprocal(rs[:, xc:xc + xcw], pacc[:, 3, 0:xcw])
                    rsb = (
                        rs[:, xc:xc + xcw].unsqueeze(1).broadcast_to([RT, C, xcw])
                    )
                    nc.vector.tensor_tensor(
                        ost[:, :, xc:xc + xcw], pacc[:, 0:3, 0:xcw], rsb,
                        mybir.AluOpType.mult,
                    )

                pending_store.append((y0, ost))

            flush_store()

    nc.compile()
    return nc


def get_program():
    if "nc" not in _CACHE:
        _CACHE["nc"] = _build_program()
    return _CACHE["nc"]


def make_shards(data: np.ndarray, kernels: np.ndarray):
    """Full inputs -> per-core input maps (with halo + zero padding)."""
    data = np.asarray(data, dtype=np.float32)
    kernels = np.asarray(kernels, dtype=np.float32)
    # zero-pad data: 2 rows top/bottom, 2 cols left, 6 cols right;
    # row-major layouts: data [row, c, x], kern [row, tap, x]
    dpad = np.zeros((B, H + 2 * HALO, C, WP), dtype=ml_dtypes.bfloat16)
    dpad[:, HALO:HALO + H, :, HALO:HALO + W] = (
        data.transpose(0, 2, 1, 3).astype(ml_dtypes.bfloat16)
    )
    in_maps = []
    for core in range(NCORES):
        b, hh = divmod(core, 2)
        r0 = hh * HS
        dsh = np.ascontiguousarray(dpad[b, r0:r0 + HS + 2 * HALO])
        ksh = np.zeros((KH, KW * KW, W), dtype=ml_dtypes.bfloat16)
        ksh[KROWPAD:KROWPAD + HS] = (
            kernels[b, :, r0:r0 + HS, :].transpose(1, 0, 2)
            .astype(ml_dtypes.bfloat16)
        )
        in_maps.append({"data": dsh, "kern": ksh})
    return in_maps


def assemble(results) -> np.ndarray:
    out = np.empty((B, C, H, W), dtype=np.float32)
    for core in range(NCORES):
        b, hh = divmod(core, 2)
        out[b, :, hh * HS:(hh + 1) * HS, :] = (
            results[core]["out"].astype(np.float32).transpose(1, 0, 2)
        )
    return out


def kernel(data: np.ndarray, kernels: np.ndarray) -> np.ndarray:
    from concourse.bass_utils import run_bass_kernel_spmd

    nc = get_program()
    in_maps = make_shards(data, kernels)
    res = run_bass_kernel_spmd(nc, in_maps, list(range(NCORES)))
    return assemble(res.results)


if __name__ == "__main__":
    get_program()
    print("program built OK")


# revision 5
# speedup vs baseline: 1.3591x; 1.3591x over previous
"""Trainium2 Bass kernel: per-pixel 5x5 kernel application (KPN-style).

    out[b,c,y,x] = sum_{i,j} softmax(kernels[b,:,y,x])[i*5+j]
                   * zpad(data)[b,c,y+i,x+j]          (i,j in 0..4, r=2)

Sharding (8 NeuronCores, pure data parallel, no collectives):
    core = (b, H-half): 4 batches x 2 row-bands of 360 rows.
    Halo rows come from overlapping host-side slices of the full input.

The per-core HBM pipe sustains only ~92 GB/s regardless of DMA engine
spreading, so runtime is dominated by bytes moved. Traffic reduction:
    - kernel tensor ships as int8 with a per-(row, di-group) affine
      dequant (scale/bias), applied FOR FREE inside the ACT exp
      (exp(scale*k + bias)); 23MB -> 11.5MB. Measured rel-l2 vs f32
      reference: 8.1e-3 (gate 2e-2).
    - data ships bf16; output stores bf16 (upcast on host).
    Total ~17.2MB/core -> ~187us DMA floor at 92GB/s.

Compute (overlapped under the DMA stream):
    - kernel taps arrive as 5 di-group DMAs per row-tile (rows shifted
      by -di, 5 dj taps DRAM-contiguous: 124 x 6.4KB descriptors) on the
      software DGE queue, which nothing else blocks.
    - ACT: exp per (x-chunk, di-group) with int8 in, bf16 out, dequant
      scale/bias as per-partition operands.
    - DVE: tap products q = e * d in bf16 2x; one instruction covers the
      even (or odd) dj taps of a group via overlapping-window APs; two
      parity copies of the data keep operands 4B-aligned.
    - PE: stationary shift matrix S_di[k,m] = [k == m+di] undoes the
      load shift; accumulates 25 q planes per channel + 25 exp planes
      (softmax denominator) into PSUM.
    - DVE: out_c = PSUM_c * reciprocal(PSUM_sum) -> bf16.

Queue assignment (stall avoidance): SWDGE carries only kq loads; the
scalar ring carries data/scale loads (issued before that engine's exps);
the sync ring carries stores (the sync engine runs nothing else, so
store semaphore waits never block loads or compute).

kernel(**inputs) takes the FULL inputs and returns the FULL output.
"""

import numpy as np
import ml_dtypes

B, C, H, W, KW = 4, 3, 720, 1280, 5
NCORES = 8
HS = H // 2            # 360 output rows per shard
RT = 120               # output rows per row-tile
NRT = HS // RT         # 3 row-tiles
HALO = 2
DP = RT + 2 * HALO     # 124 partitions (data space)
WP = 1288              # padded data width: 2 left + 1280 + 6 right
KROWPAD = 4            # zero rows around each kernel shard (top+bottom)
KH = HS + 2 * KROWPAD  # 368
XCH = [(0, 512), (512, 512), (1024, 256)]

RECIP_ACT = False      # reciprocal via ACT ln/exp instead of DVE Newton

_CACHE = {}


def _build_program():
    import concourse.bacc as bacc
    import concourse.mybir as mybir
    from concourse.bass import AP
    from concourse import tile

    f32 = mybir.dt.float32
    bf16 = mybir.dt.bfloat16
    i8 = mybir.dt.int8

    nc = bacc.Bacc(
        "TRN2",
        target_bir_lowering=False,
        debug=False,
        enable_asserts=False,
        num_devices=NCORES,
    )
    d_data = nc.dram_tensor("data", [HS + 2 * HALO, C, WP], bf16, kind="ExternalInput")
    d_kq = nc.dram_tensor("kq", [KH, KW * KW, W], i8, kind="ExternalInput")
    d_scb = nc.dram_tensor("scb", [KH, KW, 2], f32, kind="ExternalInput")
    d_out = nc.dram_tensor("out", [HS, C, W], bf16, kind="ExternalOutput")

    # Shift matrices S_di[k, m] = 1 iff k == m + di  (k: 124 data rows,
    # m: 120 out rows). Baked into the NEFF as a Const tensor.
    s_np = np.zeros((KW, DP, RT), dtype=ml_dtypes.bfloat16)
    for di in range(KW):
        for m in range(RT):
            s_np[di, m + di, m] = 1.0
    d_s = nc.inline_tensor(np.ascontiguousarray(s_np), "smat")

    KROW = KW * KW * W  # element stride between rows of d_kq

    with tile.TileContext(nc) as tc:
        with tc.tile_pool(name="const", bufs=1) as cpool, \
             tc.tile_pool(name="dbf", bufs=2) as dbfpool, \
             tc.tile_pool(name="kq", bufs=7) as kqpool, \
             tc.tile_pool(name="scb", bufs=7) as scbpool, \
             tc.tile_pool(name="ech", bufs=12) as epool, \
             tc.tile_pool(name="qt", bufs=3) as qpool, \
             tc.tile_pool(name="fin", bufs=2) as fpool, \
             tc.tile_pool(name="ps", bufs=2, space="PSUM") as ppool:

            s_sb = cpool.tile([DP, KW, RT], bf16)
            nc.sync.dma_start(out=s_sb[:], in_=d_s.ap().transpose([1, 0, 2]))

            for rt in range(NRT):
                y0 = rt * RT

                # kernel di-group tiles, rows shifted by -di:
                # kq[di][p, dj, x] = kq8[y0 + p - di, 5*di + dj, x];
                # one SWDGE DMA per group, 124 x 6.4KB descriptors.
                kqs, scbs = [], []
                for di in range(KW):
                    kq = kqpool.tile([DP, KW, W], i8, tag="kq")
                    off = (KROWPAD + y0 - di) * KROW + di * KW * W
                    nc.gpsimd.dma_start(
                        out=kq[:],
                        in_=AP(d_kq, off, [[KROW, DP], [1, KW * W]]),
                    )
                    kqs.append(kq)
                    scb = scbpool.tile([DP, 2], f32, tag="scb")
                    soff = (KROWPAD + y0 - di) * KW * 2 + di * 2
                    nc.scalar.dma_start(
                        out=scb[:],
                        in_=AP(d_scb, soff, [[KW * 2, DP], [1, 2]]),
                    )
                    scbs.append(scb)
                    if di == 0:
                        # data rows y0-2 .. y0+121 (host-padded), bf16
                        dbf0 = dbfpool.tile([DP, C, WP], bf16, tag="dbf0")
                        dbf1 = dbfpool.tile([DP, C, WP], bf16, tag="dbf1")
                        nc.scalar.dma_start(
                            out=dbf0[:], in_=d_data.ap()[y0:y0 + DP],
                        )
                # dbf1 = dbf0 shifted one element left (odd-dj 4B alignment);
                # tensor_copy runs in DVE 4x mode.
                f0 = dbf0[:].rearrange("p c w -> p (c w)")
                f1 = dbf1[:].rearrange("p c w -> p (c w)")
                nc.vector.tensor_copy(f1[:, 0:C * WP - 1], f0[:, 1:C * WP])

                rs = fpool.tile([RT, W], f32, tag="rs", bufs=1)
                ost = fpool.tile([RT, C, W], bf16, tag="ost")

                dbf0_ap = dbf0[:]
                dbf1_ap = dbf1[:]
                dp_stride = dbf0_ap.ap[0][0]

                for (xc, xcw) in XCH:
                    # PSUM banks: 0..2 = channel accumulators, 3 = sumexp
                    pacc = ppool.tile([RT, 4, 512], f32, tag="pacc")

                    ech = []
                    for di in range(KW):
                        e = epool.tile([DP, KW, 512], bf16, tag="ech")
                        nc.scalar.activation(
                            e[:, :, 0:xcw],
                            kqs[di][:, :, xc:xc + xcw],
                            mybir.ActivationFunctionType.Exp,
                            bias=scbs[di][:, 1:2],
                            scale=scbs[di][:, 0:1],
                        )
                        ech.append(e)

                    for di in range(KW):
                        e = ech[di]
                        lhs = s_sb[:, di, :]
                        first = di == 0
                        last = di == KW - 1
                        # sumexp: stream the exp planes directly
                        for dj in range(KW):
                            nc.tensor.matmul(
                                out=pacc[:, 3, 0:xcw],
                                lhsT=lhs,
                                rhs=e[:, dj, 0:xcw],
                                start=first and dj == 0,
                                stop=last and dj == KW - 1,
                            )
                        # tap products q[p, dj, c, x] = e[p, dj, x] *
                        # d[p, c, x + dj]; one DVE instruction per parity
                        # (overlapping dj windows, stride 2, on dbf0/dbf1).
                        qt = qpool.tile([DP, KW, C, 512], bf16, tag="qt")
                        e_ev = (
                            e[:, 0:KW:2, 0:xcw]
                            .unsqueeze(2).broadcast_to([DP, 3, C, xcw])
                        )
                        d_ev = AP(
                            dbf0_ap.tensor,
                            dbf0_ap.offset + xc,
                            [[dp_stride, DP], [2, 3], [WP, C], [1, xcw]],
                        )
                        nc.vector.tensor_tensor(
                            qt[:, 0:KW:2, :, 0:xcw], e_ev, d_ev,
                            mybir.AluOpType.mult,
                        )
                        e_od = (
                            e[:, 1:KW:2, 0:xcw]
                            .unsqueeze(2).broadcast_to([DP, 2, C, xcw])
                        )
                        d_od = AP(
                            dbf1_ap.tensor,
                            dbf1_ap.offset + xc,
                            [[dp_stride, DP], [2, 2], [WP, C], [1, xcw]],
                        )
                        nc.vector.tensor_tensor(
                            qt[:, 1:KW:2, :, 0:xcw], e_od, d_od,
                            mybir.AluOpType.mult,
                        )

                        for dj in range(KW):
                            for c in range(C):
                                nc.tensor.matmul(
                                    out=pacc[:, c, 0:xcw],
                                    lhsT=lhs,
                                    rhs=qt[:, dj, c, 0:xcw],
                                    start=first and dj == 0,
                                    stop=last and dj == KW - 1,
                                )

                    if RECIP_ACT:
                        lnt = fpool.tile([RT, 512], f32, tag="lnt")
                        nc.scalar.activation(
                            lnt[:, 0:xcw], pacc[:, 3, 0:xcw],
                            mybir.ActivationFunctionType.Ln,
                        )
                        nc.scalar.activation(
                            rs[:, xc:xc + xcw], lnt[:, 0:xcw],
                            mybir.ActivationFunctionType.Exp,
                            scale=-1.0,
                        )
                    else:
                        nc.vector.reciprocal(rs[:, xc:xc + xcw], pacc[:, 3, 0:xcw])
                    rsb = (
                        rs[:, xc:xc + xcw].unsqueeze(1).broadcast_to([RT, C, xcw])
                    )
                    nc.vector.tensor_tensor(
                        ost[:, :, xc:xc + xcw], pacc[:, 0:3, 0:xcw], rsb,
                        mybir.AluOpType.mult,
                    )

                nc.sync.dma_start(out=d_out.ap()[y0:y0 + RT], in_=ost[:])

    nc.compile()
    return nc


def get_program():
    if "nc" not in _CACHE:
        _CACHE["nc"] = _build_program()
    return _CACHE["nc"]


def make_shards(data: np.ndarray, kernels: np.ndarray):
    """Full inputs -> per-core input maps (quantized kernels + halo pad)."""
    data = np.asarray(data, dtype=np.float32)
    kernels = np.asarray(kernels, dtype=np.float32)
    # zero-pad data: 2 rows top/bottom, 2 cols left, 6 cols right;
    # row-major layouts: data [row, c, x], kern [row, tap, x]
    dpad = np.zeros((B, H + 2 * HALO, C, WP), dtype=ml_dtypes.bfloat16)
    dpad[:, HALO:HALO + H, :, HALO:HALO + W] = (
        data.transpose(0, 2, 1, 3).astype(ml_dtypes.bfloat16)
    )
    # int8 affine quantization per (b, di-group, row): k ~ s*q + bb
    kg = kernels.reshape(B, KW, KW, H, W)
    mx = kg.max(axis=(2, 4))                        # [B, KW, H]
    mn = kg.min(axis=(2, 4))
    s = np.maximum((mx - mn) / 255.0, 1e-30)
    q = np.clip(
        np.rint((kg - mn[:, :, None, :, None]) / s[:, :, None, :, None]) - 128.0,
        -128, 127,
    ).astype(np.int8)                               # [B, KW, KW, H, W]
    bb = mn + 128.0 * s                             # k ~ s*q + bb
    in_maps = []
    for core in range(NCORES):
        b, hh = divmod(core, 2)
        r0 = hh * HS
        dsh = np.ascontiguousarray(dpad[b, r0:r0 + HS + 2 * HALO])
        kq = np.zeros((KH, KW * KW, W), dtype=np.int8)
        kq[KROWPAD:KROWPAD + HS] = (
            q[b].reshape(KW * KW, H, W)[:, r0:r0 + HS, :].transpose(1, 0, 2)
        )
        scb = np.zeros((KH, KW, 2), dtype=np.float32)
        scb[KROWPAD:KROWPAD + HS, :, 0] = s[b, :, r0:r0 + HS].T
        scb[KROWPAD:KROWPAD + HS, :, 1] = bb[b, :, r0:r0 + HS].T
        in_maps.append({"data": dsh, "kq": kq, "scb": scb})
    return in_maps


def assemble(results) -> np.ndarray:
    out = np.empty((B, C, H, W), dtype=np.float32)
    for core in range(NCORES):
        b, hh = divmod(core, 2)
        out[b, :, hh * HS:(hh + 1) * HS, :] = (
            results[core]["out"].astype(np.float32).transpose(1, 0, 2)
        )
    return out


def kernel(data: np.ndarray, kernels: np.ndarray) -> np.ndarray:
    from concourse.bass_utils import run_bass_kernel_spmd

    nc = get_program()
    in_maps = make_shards(data, kernels)
    res = run_bass_kernel_spmd(nc, in_maps, list(range(NCORES)))
    return assemble(res.results)


if __name__ == "__main__":
    get_program()
    print("program built OK")
